# revision 3
# baseline (speedup 1.0000x reference)
"""CMoE hash-routed expert FFN on 8 NeuronCores (expert-parallel).

Host side (the shard/unshard steps): compute hash routing
e = (token_id % 5099) % 64, first-come slot assignment with capacity 512,
scatter tokens into a per-expert [E, D, C] buffer (transposed, bf16), and
shard 8 experts to each of the 8 cores along with that core's (transposed,
bf16) expert weights.  Device side: per expert
    h  = relu(A @ Wk^T)^2        [C, F]
    kv = h @ Wv^T                [C, D]
    r  = sigmoid(A @ Wr^T)       [C, D]
    out = r * kv
computed entirely in transposed form (contraction dim on SBUF partitions),
bf16 matmul operands with fp32 PSUM accumulation.  Host gathers each
token's slot back out of [E, D, C] and zeroes dropped tokens.
"""

import numpy as np
import ml_dtypes

import concourse.bass as bass
import concourse.mybir as mybir
import concourse.tile as tile
from concourse import bacc
from concourse.bass import ts
from concourse.bass_utils import run_bass_kernel_spmd

HASH_PRIME = 5099
B, T, D, F, E = 8, 4096, 512, 1792, 64
S = B * T
C = 512  # capacity = max(4, ceil(S/E))
N_CORES = 8
E_LOC = E // N_CORES  # experts per core

BF16 = mybir.dt.bfloat16
F32 = mybir.dt.float32

_NC = None  # cached compiled Bass program
LAST_RESULT = None  # BassKernelResults of the most recent run (for test.py)


def _build_nc(e_loc=E_LOC, d=D, f=F, c=C):
    """One SPMD program: each core computes e_loc experts' FFN."""
    kd = d // 128   # contraction tiles over D
    kf = f // 128   # contraction tiles over F
    nc = bacc.Bacc("TRN2", target_bir_lowering=False, debug=False,
                   num_devices=N_CORES)

    a_t = nc.dram_tensor("a_t", [e_loc, d, c], BF16, kind="ExternalInput")
    wk_t = nc.dram_tensor("wk_t", [e_loc, d, f], BF16, kind="ExternalInput")
    wr_t = nc.dram_tensor("wr_t", [e_loc, d, d], BF16, kind="ExternalInput")
    wv_t = nc.dram_tensor("wv_t", [e_loc, f, d], BF16, kind="ExternalInput")
    out_t = nc.dram_tensor("out_t", [e_loc, d, c], F32, kind="ExternalOutput")

    with tile.TileContext(nc) as tc:
        with (
            tc.tile_pool(name="wts", bufs=2) as wts,
            tc.tile_pool(name="acts", bufs=2) as acts,
            tc.tile_pool(name="ph", bufs=2, space="PSUM") as ph,
            tc.tile_pool(name="pr", bufs=2, space="PSUM") as pr,
            tc.tile_pool(name="pkv", bufs=2, space="PSUM") as pkv,
        ):
            for e in range(e_loc):
                at = wts.tile([128, kd, c], BF16, tag="at")
                wk = wts.tile([128, kd, f], BF16, tag="wk")
                wr = wts.tile([128, kd, d], BF16, tag="wr")
                wv = wts.tile([128, kf, d], BF16, tag="wv")
                # two HWDGE rings: sync (at, wv) + scalar (wk, wr); one ring
                # alone (~190 GB/s) can't stay ahead of the PE stream
                a_src = a_t[e].rearrange("(ko p) c -> p ko c", p=128)
                k_src = wk_t[e].rearrange("(ko p) f -> p ko f", p=128)
                if e == 0:
                    # per-k-tile chunks so the first matmul group only waits
                    # for ~0.7MB instead of the whole expert's loads
                    for kt in range(kd):
                        nc.sync.dma_start(at[:, kt, :], a_src[:, kt, :])
                        nc.scalar.dma_start(wk[:, kt, :], k_src[:, kt, :])
                else:
                    nc.sync.dma_start(at[:], a_src)
                    nc.scalar.dma_start(wk[:], k_src)
                nc.scalar.dma_start(wr[:], wr_t[e].rearrange("(ko p) g -> p ko g", p=128))
                nc.sync.dma_start(wv[:], wv_t[e].rearrange("(fo p) g -> p fo g", p=128))

                # h^T[f, c] = (relu(Wk^T.T @ A^T))^2, kept bf16 for matmul 2
                hb = acts.tile([128, kf, c], BF16, tag="hb")
                for ft in range(kf):
                    psum_h = ph.tile([128, c], F32, tag="psh")
                    for kt in range(kd):
                        nc.tensor.matmul(
                            psum_h[:],
                            lhsT=wk[:, kt, ts(ft, 128)],
                            rhs=at[:, kt, :],
                            start=(kt == 0),
                            stop=(kt == kd - 1),
                        )
                    nc.scalar.activation(hb[:, ft, :], psum_h[:],
                                         mybir.ActivationFunctionType.Relu)
                    nc.vector.tensor_mul(hb[:, ft, :], hb[:, ft, :], hb[:, ft, :])

                # r^T[g, c] = sigmoid(Wr^T.T @ A^T), fp32 in SBUF
                sig = acts.tile([128, kd, c], F32, tag="sig")
                for gt in range(kd):
                    psum_r = pr.tile([128, c], F32, tag="psr")
                    for kt in range(kd):
                        nc.tensor.matmul(
                            psum_r[:],
                            lhsT=wr[:, kt, ts(gt, 128)],
                            rhs=at[:, kt, :],
                            start=(kt == 0),
                            stop=(kt == kd - 1),
                        )
                    nc.scalar.activation(sig[:, gt, :], psum_r[:],
                                         mybir.ActivationFunctionType.Sigmoid)

                # kv^T[dd, c] = Wv^T.T @ h^T ; out = sig * kv
                ob = acts.tile([128, kd, c], F32, tag="ob")
                for dt in range(kd):
                    psum_kv = pkv.tile([128, c], F32, tag="pskv")
                    for ft in range(kf):
                        nc.tensor.matmul(
                            psum_kv[:],
                            lhsT=wv[:, ft, ts(dt, 128)],
                            rhs=hb[:, ft, :],
                            start=(ft == 0),
                            stop=(ft == kf - 1),
                        )
                    nc.vector.tensor_mul(ob[:, dt, :], psum_kv[:], sig[:, dt, :])
                    # store each d-tile as it finishes, on the idle SWDGE ring
                    nc.gpsimd.dma_start(
                        out_t[e].rearrange("(ko p) c -> p ko c", p=128)[:, dt, :],
                        ob[:, dt, :])

    nc.compile()
    return nc


def _route(token_ids):
    tid = token_ids.reshape(S).astype(np.int64)
    e_idx = (tid % HASH_PRIME) % E
    order = np.argsort(e_idx, kind="stable")
    sorted_e = e_idx[order]
    starts = np.searchsorted(sorted_e, np.arange(E))
    pos = np.empty(S, np.int64)
    pos[order] = np.arange(S) - starts[sorted_e]
    kept = pos < C
    return e_idx, pos, kept


def kernel(x, token_ids, Wk, Wr, Wv):
    global _NC, LAST_RESULT
    if _NC is None:
        _NC = _build_nc()

    e_idx, pos, kept = _route(token_ids)

    bf16 = ml_dtypes.bfloat16
    xf = np.ascontiguousarray(x, dtype=np.float32).reshape(S, D)
    disp_t = np.zeros((E, D, C), np.float32)
    disp_t[e_idx[kept], :, pos[kept]] = xf[kept]
    a_t = disp_t.astype(bf16)

    wk_t = np.asarray(Wk, dtype=np.float32).transpose(0, 2, 1).astype(bf16)
    wr_t = np.asarray(Wr, dtype=np.float32).transpose(0, 2, 1).astype(bf16)
    wv_t = np.asarray(Wv, dtype=np.float32).transpose(0, 2, 1).astype(bf16)

    in_maps = [
        {
            "a_t": a_t[i * E_LOC:(i + 1) * E_LOC],
            "wk_t": wk_t[i * E_LOC:(i + 1) * E_LOC],
            "wr_t": wr_t[i * E_LOC:(i + 1) * E_LOC],
            "wv_t": wv_t[i * E_LOC:(i + 1) * E_LOC],
        }
        for i in range(N_CORES)
    ]

    LAST_RESULT = run_bass_kernel_spmd(_NC, in_maps, list(range(N_CORES)))
    out_t = np.concatenate(
        [LAST_RESULT.results[i]["out_t"] for i in range(N_CORES)], axis=0)

    yf = out_t[e_idx, :, np.minimum(pos, C - 1)]
    yf[~kept] = 0.0
    return np.ascontiguousarray(yf.reshape(B, T, D), dtype=np.float32)
